# revision 29
# baseline (speedup 1.0000x reference)
"""HeteroClassifier GNN kernel for 8 TRN2 NeuronCores (Bass/Tile).

Sharding: L1 edges by dst node-range (owner core aggregates its nodes);
L2 edges by src node-range (gather tables stay core-local); per-core
[B,2] partial outputs are summed to unshard. Host does structure-only
prep (degree counts, edge grouping, padded window/stream layouts, index
maps). All value compute runs on the NeuronCores.

Gather mechanisms (HW-validated):
- L1 edge gather: per-column indirect DMA (one offset per partition).
- x realign and L2 edge gather: gpsimd ap_gather from an SBUF-replicated
  bf16 table; each 16-partition group processes its own edge stream with
  a masked coefficient table, then windowed segment reduction.
"""

import numpy as np

import concourse.bass as bass
import concourse.bacc as bacc
import concourse.mybir as mybir
import concourse.tile as tile
from concourse import bass_utils

LAST_EXEC_NS = -1
LAST_TRACE = None
N = 200000
R = 4
E = 1000000
B = 1024
NCORES = 8
P = 128
G16 = 16          # partitions per gpsimd group
NG = P // G16     # 8 groups


def _sizes():
    nb = N // NCORES
    nwin1 = (nb + P - 1) // P
    nwin2 = (B + P - 1) // P
    return nb, nwin1, nwin2


def _plan_and_pack(keys_by_core, nitems, nwin, src_by_core, vals_by_core):
    """Window packing for per-column DMA (L1). keys: per-core local group
    key per edge. Returns (ls, ni, per-core (idx [P,ni] i32,
    [vals [P,ni] f32...], order))."""
    orders, cnts_o, edata = [], [], []
    for c in range(NCORES):
        keys = keys_by_core[c]
        cnts = np.bincount(keys, minlength=nitems)
        order = np.argsort(-cnts, kind="stable")
        rp = np.empty(nitems, dtype=np.int64)
        rp[order] = np.arange(nitems)
        six = np.argsort(keys, kind="stable")
        ks = keys[six]
        starts = np.searchsorted(ks, np.arange(nitems))
        j_in_grp = np.arange(len(ks)) - starts[ks]
        orders.append(order)
        cnts_o.append(cnts[order])
        edata.append((six, ks, j_in_grp, rp))
    ls = []
    for k in range(nwin):
        m = 1
        for c in range(NCORES):
            seg = cnts_o[c][k * P:(k + 1) * P]
            if len(seg):
                m = max(m, int(seg.max()))
        ls.append(m)
    ni = int(np.sum(ls))
    wbase = np.concatenate([[0], np.cumsum(ls)]).astype(np.int64)
    packed = []
    for c in range(NCORES):
        six, ks, j_in_grp, rp = edata[c]
        rank = rp[ks]
        lane = rank % P
        win = rank // P
        col = wbase[win] + j_in_grp
        idx = np.zeros((P, ni), dtype=np.int32)
        idx[lane, col] = src_by_core[c][six]
        vals = []
        for va in vals_by_core[c]:
            v = np.zeros((P, ni), dtype=np.float32)
            v[lane, col] = va[six]
            vals.append(v)
        packed.append((idx, vals, orders[c]))
    return ls, ni, packed


def _pack_apg(lane, win, row, coef, nwin, seg_len):
    """Pack an edge stream for ap_gather with group-shared indices.

    lane/win: target slot (lane in [0,P), win in [0,nwin)); row: int16
    table row to gather; coef: f32 coefficient (applied at target lane).
    seg_len[w]: shared per-window segment length (max over groups/cores).
    Returns (idx_tile [P, S//16] i16, coeff [P, S] f32) with
    S = 16*ceil(sum(seg_len)/16)."""
    S0 = int(np.sum(seg_len))
    S = ((S0 + 15) // 16) * 16
    sbase = np.concatenate([[0], np.cumsum(seg_len)]).astype(np.int64)
    g = lane // G16
    c16 = lane % G16
    # position within (group, window)
    order = np.lexsort((np.arange(len(lane)), win, g))
    gs, ws = g[order], win[order]
    # j_in_seg: running index within each (g, w) bucket
    key = gs * nwin + ws
    ksorted = key  # already sorted by (g, w)
    starts = np.searchsorted(ksorted, np.arange(NG * nwin))
    j_in = np.arange(len(ksorted)) - starts[ksorted]
    jpos = sbase[ws] + j_in
    import ml_dtypes
    idx_tile = np.zeros((P, S // 16), dtype=np.int16)
    coeff = np.zeros((P, S), dtype=ml_dtypes.bfloat16)
    rs, cs, ls_ = row[order], coef[order], c16[order]
    idx_tile[gs * G16 + (jpos % 16), jpos // 16] = rs
    coeff[gs * G16 + ls_, jpos] = cs
    return idx_tile, coeff, S


def _prep(feat, src, dst, ew, graph_ids):
    nb, nwin1, nwin2 = _sizes()
    src = np.asarray(src); dst = np.asarray(dst)
    ew = np.asarray(ew); gid = np.asarray(graph_ids)

    od = np.stack([np.bincount(src[r], minlength=N) for r in range(R)])
    idg = np.stack([np.bincount(dst[r], minlength=N) for r in range(R)])
    cnt = np.bincount(gid, minlength=B)
    ods = (1.0 / np.sqrt(np.clip(od, 1, None))).astype(np.float32)
    ids = (1.0 / np.sqrt(np.clip(idg, 1, None))).astype(np.float32)
    qn = (ids / np.clip(cnt, 1, None)[gid][None, :]).astype(np.float32)

    meta = {"L1": [], "L2": [], "X": {}}
    per_core = [dict() for _ in range(NCORES)]

    # ---- L1: dst-sharded, per-column window packing; gather from feat
    orders1 = [[None] * NCORES for _ in range(R)]
    for r in range(R):
        core_of = dst[r] // nb
        keys, srcs, vals = [], [], []
        for c in range(NCORES):
            m = core_of == c
            keys.append((dst[r][m] - c * nb).astype(np.int64))
            srcs.append(src[r][m])
            vals.append([ew[r][m], ods[r][src[r][m]]])
        ls, ni, packed = _plan_and_pack(keys, nb, nwin1, srcs, vals)
        meta["L1"].append({"ls": ls, "ni": ni})
        for c in range(NCORES):
            idx, (vew, vos), order = packed[c]
            orders1[r][c] = order
            per_core[c][f"l1idx_{r}"] = idx
            per_core[c][f"l1ew_{r}"] = vew
            per_core[c][f"l1os_{r}"] = vos
            lo = c * nb
            on = np.zeros(nwin1 * P, dtype=np.float32)
            on[:nb] = ods[r, lo:lo + nb]
            per_core[c][f"odsl_{r}"] = on.reshape(nwin1, P).T.copy()

    # ---- x realign via ap_gather: natural (p,k) <- aggR rank row,
    # coeff = ids (folds the rid multiply); absent nodes coeff 0
    SX = nwin1 * G16
    meta["X"] = {"S": SX}
    for r in range(R):
        for c in range(NCORES):
            order = orders1[r][c]
            rp = np.empty(nb, dtype=np.int64)
            rp[order] = np.arange(nb)
            lo = c * nb
            degl = np.bincount(
                dst[r][(dst[r] >= lo) & (dst[r] < lo + nb)] - lo, minlength=nb)
            n_nat = np.arange(nb)
            lane = (n_nat % P).astype(np.int64)
            win = (n_nat // P).astype(np.int64)
            rr = rp[n_nat]
            row = ((rr % P) * nwin1 + rr // P).astype(np.int64)
            coefv = np.where(degl > 0, ids[r, lo:lo + nb], 0.0) \
                .astype(np.float32)
            idx_t, coeff, S = _pack_apg(
                lane, win, row, coefv, nwin1,
                np.full(nwin1, G16, dtype=np.int64))
            assert S == SX
            per_core[c][f"xidx_{r}"] = idx_t
            per_core[c][f"xcoef_{r}"] = coeff

    # ---- L2: src-sharded, ap_gather streams grouped by graph rank
    # shared graph ranking per (relation): by per-core counts is fine but
    # ranks must be shared across cores? gpos handles per-core; use global
    # per-relation ranking by total count so window segments are shared.
    for r in range(R):
        gcnt = np.bincount(gid[dst[r]], minlength=B)
        gorder = np.argsort(-gcnt, kind="stable")
        grp = np.empty(B, dtype=np.int64)
        grp[gorder] = np.arange(B)
        core_of = src[r] // nb
        # shared segment lengths: max over cores/groups per window
        seg = np.zeros(nwin2, dtype=np.int64)
        percore_dat = []
        for c in range(NCORES):
            m = core_of == c
            d = dst[r][m]
            rk = grp[gid[d]]
            lane = rk % P
            win = rk // P
            n = src[r][m] - c * nb
            row = (n % P) * nwin1 + n // P
            coefv = qn[r][d]
            percore_dat.append((lane, win, row, coefv))
            cnts = np.bincount((lane // G16) * nwin2 + win,
                               minlength=NG * nwin2)
            seg = np.maximum(seg, cnts.reshape(NG, nwin2).max(axis=0))
        S2 = int(((seg.sum() + 15) // 16) * 16)
        meta["L2"].append({"seg": seg.tolist(), "S": S2})
        for c in range(NCORES):
            lane, win, row, coefv = percore_dat[c]
            idx_t, coeff, S = _pack_apg(lane, win, row.astype(np.int64),
                                        coefv, nwin2, seg)
            assert S == S2
            per_core[c][f"l2idx_{r}"] = idx_t
            per_core[c][f"l2coef_{r}"] = coeff
        # gpos: natural graph b=(k*P+p) -> rank row (lane-major) in prank
        for c in range(NCORES):
            pos = np.full(nwin2 * P, nwin2 * P, dtype=np.int32)
            bb = np.arange(B)
            rk = grp[bb]
            pos[bb] = ((rk % P) * nwin2 + rk // P).astype(np.int32)
            per_core[c][f"gpos_{r}"] = pos.reshape(nwin2, P).T.copy()

    return per_core, meta


def _build_program(meta):
    nb, nwin1, nwin2 = _sizes()
    nc = bacc.Bacc("TRN2", target_bir_lowering=False, debug=False,
                   num_devices=NCORES)
    f32, i32 = mybir.dt.float32, mybir.dt.int32
    bf16, i16 = mybir.dt.bfloat16, mybir.dt.int16
    AL = mybir.AluOpType
    SX = meta["X"]["S"]

    feat = nc.dram_tensor("feat", [N, 2], f32, kind="ExternalInput").ap()
    w1p = nc.dram_tensor("w1p", [P, 16 * 2 * R], f32, kind="ExternalInput").ap()
    b1b = nc.dram_tensor("b1b", [P, R * 16], f32, kind="ExternalInput").ap()
    W2 = nc.dram_tensor("W2", [R, 16, 16], f32, kind="ExternalInput").ap()
    b2b = nc.dram_tensor("b2b", [P, R * 16], f32, kind="ExternalInput").ap()
    Wc = nc.dram_tensor("Wc", [16, 2], f32, kind="ExternalInput").ap()
    bc = nc.dram_tensor("bc", [2], f32, kind="ExternalInput").ap()
    ins = {}
    for r in range(R):
        ni1 = meta["L1"][r]["ni"]
        S2 = meta["L2"][r]["S"]
        for nm, shp, dt in (
            (f"l1idx_{r}", [P, ni1], i32), (f"l1ew_{r}", [P, ni1], f32),
            (f"l1os_{r}", [P, ni1], f32), (f"odsl_{r}", [P, nwin1], f32),
            (f"xidx_{r}", [P, SX // 16], i16), (f"xcoef_{r}", [P, SX], bf16),
            (f"l2idx_{r}", [P, S2 // 16], i16), (f"l2coef_{r}", [P, S2], bf16),
            (f"gpos_{r}", [P, nwin2], i32),
        ):
            ins[nm] = nc.dram_tensor(nm, shp, dt, kind="ExternalInput").ap()
    aggB = [nc.dram_tensor(f"aggB_{r}", [nwin1 * P * 2], bf16, kind="Internal").ap()
            for r in range(R)]
    gtabB = [nc.dram_tensor(f"gB_{r}", [nwin1 * P * 2], bf16, kind="Internal").ap()
             for r in range(R)]
    prank = [nc.dram_tensor(f"prank_{r}", [(nwin2 + 1) * P, 2], f32, kind="Internal").ap()
             for r in range(R)]
    out_part = nc.dram_tensor("out_part", [B, 2], f32, kind="ExternalOutput").ap()
    bias_out = nc.dram_tensor("bias_out", [1, 2], f32, kind="ExternalOutput").ap()

    NE1 = nwin1 * P  # 25088 table rows

    def reduce_windows(ga, out_t, ls, nwin):
        col = 0
        k = 0
        while k < nwin:
            k2 = k
            while k2 < nwin and ls[k2] == ls[k]:
                k2 += 1
            lk, nk = ls[k], k2 - k
            seg = ga[:, col:col + nk * lk, :].rearrange(
                "p (n l) c -> p n c l", l=lk)
            nc.vector.tensor_reduce(out=out_t[:, k:k2, :], in_=seg,
                                    op=AL.add, axis=mybir.AxisListType.X)
            col += nk * lk
            k = k2

    with tile.TileContext(nc) as tc:
        with (tc.tile_pool(name="glob", bufs=1) as gpool,
              tc.tile_pool(name="psum", bufs=2, space="PSUM") as psum):
            zt = gpool.tile([P, 2], f32, name="zt")
            nc.vector.memset(zt[:], 0.0)
            for r in range(R):
                nc.sync.dma_start(out=prank[r][nwin2 * P:, :], in_=zt[:])

            pr_ts = []

            # ---- phase 1: L1 per-column gathers, windowed reduce,
            # aggregates written as bf16 rank tables (lane-major rows)
            with tc.tile_pool(name="l1", bufs=2) as l1p:
                for r in range(R):
                    ni1, ls = meta["L1"][r]["ni"], meta["L1"][r]["ls"]
                    idx_t = l1p.tile([P, ni1], i32, name=f"i1_{r}", tag="i1")
                    nc.sync.dma_start(out=idx_t[:], in_=ins[f"l1idx_{r}"][:])
                    ew_t = l1p.tile([P, ni1], f32, name=f"e1_{r}", tag="e1")
                    nc.sync.dma_start(out=ew_t[:], in_=ins[f"l1ew_{r}"][:])
                    os_t = l1p.tile([P, ni1], f32, name=f"o1_{r}", tag="o1")
                    nc.sync.dma_start(out=os_t[:], in_=ins[f"l1os_{r}"][:])
                    nc.vector.tensor_tensor(out=ew_t[:], in0=ew_t[:],
                                            in1=os_t[:], op=AL.mult)
                    ga = l1p.tile([P, ni1, 2], f32, name=f"ga1_{r}", tag="ga1")
                    for c0 in range(ni1):
                        nc.gpsimd.indirect_dma_start(
                            out=ga[:, c0, :], out_offset=None, in_=feat[:],
                            in_offset=bass.IndirectOffsetOnAxis(
                                ap=idx_t[:, c0:c0 + 1], axis=0))
                    nc.vector.tensor_tensor(
                        out=ga[:, :, :], in0=ga[:, :, :],
                        in1=ew_t[:, :, None].to_broadcast([P, ni1, 2]),
                        op=AL.mult)
                    agg_t = l1p.tile([P, nwin1, 2], f32, name=f"ag_{r}",
                                     tag="ag")
                    reduce_windows(ga, agg_t, ls, nwin1)
                    agg_b = l1p.tile([P, nwin1, 2], bf16, name=f"agb_{r}",
                                     tag="agb")
                    nc.vector.tensor_copy(out=agg_b[:], in_=agg_t[:])
                    nc.sync.dma_start(
                        out=aggB[r].rearrange("(p f) -> p f", p=P),
                        in_=agg_b[:].rearrange("p k c -> p (k c)"))

            # ---- phase 2: x realign via ap_gather (coeff folds ids),
            # then h1 = relu(x@W1+b1), g tables (bf16, lane-major rows)
            with tc.tile_pool(name="p2", bufs=1) as p2:
                x_t = p2.tile([P, nwin1, 2 * R], f32, name="x_t")
                tabb = p2.tile([P, NE1, 2], bf16, name="tabb")
                go = p2.tile([P, SX, 2], bf16, name="go")
                for r in range(R):
                    nc.sync.dma_start(
                        out=tabb[:].rearrange("p n c -> p (n c)"),
                        in_=aggB[r][None, :].to_broadcast([P, NE1 * 2]))
                    xi = p2.tile([P, SX // 16], i16, name=f"xi_{r}", tag="xi")
                    nc.sync.dma_start(out=xi[:], in_=ins[f"xidx_{r}"][:])
                    xc = p2.tile([P, SX], bf16, name=f"xc_{r}", tag="xc")
                    nc.sync.dma_start(out=xc[:], in_=ins[f"xcoef_{r}"][:])
                    nc.gpsimd.ap_gather(
                        out_ap=go[:, :, :], in_ap=tabb[:, :, :],
                        idxs_ap=xi[:, :], channels=P, num_elems=NE1,
                        d=2, num_idxs=SX)
                    nc.vector.tensor_tensor(
                        out=go[:, :, :], in0=go[:, :, :],
                        in1=xc[:, :, None].to_broadcast([P, SX, 2]),
                        op=AL.mult)
                    nc.vector.tensor_reduce(
                        out=x_t[:, :, 2 * r:2 * r + 2],
                        in_=go[:, :, :].rearrange(
                            "p (k l) c -> p k c l", l=G16),
                        op=AL.add, axis=mybir.AxisListType.X)

                w1_sb = gpool.tile([P, 16 * 2 * R], f32, name="w1_sb")
                nc.sync.dma_start(out=w1_sb[:], in_=w1p[:, :])
                b1all = gpool.tile([P, R * 16], f32, name="b1all")
                nc.sync.dma_start(out=b1all[:], in_=b1b[:, :])
                b1s = gpool.tile([P, 16], f32, name="b1s")
                nc.vector.tensor_reduce(
                    out=b1s[:], in_=b1all[:].rearrange("p (r f) -> p f r", r=R),
                    op=AL.add, axis=mybir.AxisListType.X)
                h1_t = p2.tile([P, nwin1, 16], f32, name="h1_t")
                tmpV = p2.tile([P, nwin1, 16], f32, name="tmpV")
                tmpG = p2.tile([P, nwin1, 16], f32, name="tmpG")
                CR = 2 * R
                for f in range(16):
                    eng, tmp = ((nc.vector, tmpV) if f % 2 == 0
                                else (nc.gpsimd, tmpG))
                    w_ap = w1_sb[:, f * CR:(f + 1) * CR][:, None, :] \
                        .to_broadcast([P, nwin1, CR])
                    eng.tensor_tensor(out=tmp[:, :, 0:CR], in0=x_t[:, :, :],
                                      in1=w_ap, op=AL.mult)
                    nc.vector.tensor_reduce(
                        out=h1_t[:, :, f:f + 1], in_=tmp[:, :, 0:CR],
                        op=AL.add, axis=mybir.AxisListType.X)
                b_ap = b1s[:, None, :].to_broadcast([P, nwin1, 16])
                nc.vector.tensor_tensor(out=h1_t[:, :, :], in0=h1_t[:, :, :],
                                        in1=b_ap, op=AL.add)
                nc.vector.tensor_scalar_max(h1_t[:, :, :], h1_t[:, :, :], 0.0)

                wc_sb = gpool.tile([16, 2], f32, name="wc_sb")
                nc.sync.dma_start(out=wc_sb[:], in_=Wc[:, :])
                m_sb = gpool.tile([1, R * 32], f32, name="m_sb")
                ones_sb = gpool.tile([1, P], f32, name="ones_sb")
                nc.vector.memset(ones_sb[:], 1.0)
                for r in range(R):
                    w2_sb = gpool.tile([16, 16], f32, name=f"w2_{r}", tag="w2")
                    nc.sync.dma_start(out=w2_sb[:],
                                      in_=W2[r, :, :].rearrange("a b -> b a"))
                    m_ps = psum.tile([16, 2], f32, name=f"mps_{r}", tag="mps")
                    nc.tensor.matmul(out=m_ps[:], lhsT=w2_sb[:], rhs=wc_sb[:],
                                     start=True, stop=True)
                    mt = gpool.tile([16, 2], f32, name=f"mt_{r}", tag="mt")
                    nc.vector.tensor_copy(out=mt[:], in_=m_ps[:])
                    md = nc.dram_tensor(f"m_dram_{r}", [16, 2], f32,
                                        kind="Internal").ap()
                    nc.sync.dma_start(out=md[:, :], in_=mt[:])
                    nc.sync.dma_start(out=m_sb[:, r * 32:(r + 1) * 32],
                                      in_=md.rearrange("f c -> (f c)")[None, :])
                mb_ps = psum.tile([P, R * 32], f32, name="mb_ps")
                nc.tensor.matmul(out=mb_ps[:], lhsT=ones_sb[:], rhs=m_sb[:],
                                 start=True, stop=True)
                mb = gpool.tile([P, R * 32], f32, name="mb")
                nc.vector.tensor_copy(out=mb[:], in_=mb_ps[:])
                for r in range(R):
                    g_t = p2.tile([P, nwin1, 2], f32, name=f"g_{r}", tag="g")
                    for cch in range(2):
                        j = 2 * r + cch
                        tmp = tmpV if j % 2 == 0 else tmpG
                        w_ap = mb[:, r * 32:(r + 1) * 32] \
                            .rearrange("p (f c) -> p c f", c=2) \
                            [:, cch:cch + 1, :].to_broadcast([P, nwin1, 16])
                        eng = nc.vector if j % 2 == 0 else nc.gpsimd
                        eng.tensor_tensor(out=tmp[:, :, :], in0=h1_t[:, :, :],
                                          in1=w_ap, op=AL.mult)
                        nc.vector.tensor_reduce(
                            out=g_t[:, :, cch:cch + 1], in_=tmp[:, :, :],
                            op=AL.add, axis=mybir.AxisListType.X)
                    ol_t = p2.tile([P, nwin1], f32, name=f"ol_{r}", tag="ol")
                    nc.sync.dma_start(out=ol_t[:], in_=ins[f"odsl_{r}"][:])
                    g_b = p2.tile([P, nwin1, 2], bf16, name=f"gb_{r}",
                                  tag="gb")
                    nc.vector.tensor_tensor(
                        out=g_b[:, :, :], in0=g_t[:, :, :],
                        in1=ol_t[:, :, None].to_broadcast([P, nwin1, 2]),
                        op=AL.mult)
                    nc.sync.dma_start(
                        out=gtabB[r].rearrange("(p f) -> p f", p=P),
                        in_=g_b[:].rearrange("p k c -> p (k c)"))

            # ---- phase 3: L2 via ap_gather per relation
            with tc.tile_pool(name="p3", bufs=1) as p3:
                S2max = max(meta["L2"][r]["S"] for r in range(R))
                tab2 = p3.tile([P, NE1, 2], bf16, name="tab2")
                go2 = p3.tile([P, S2max, 2], bf16, name="go2")
                for r in range(R):
                    S2 = meta["L2"][r]["S"]
                    seg = meta["L2"][r]["seg"]
                    nc.sync.dma_start(
                        out=tab2[:].rearrange("p n c -> p (n c)"),
                        in_=gtabB[r][None, :].to_broadcast([P, NE1 * 2]))
                    li = p3.tile([P, S2 // 16], i16, name=f"li_{r}", tag="li")
                    nc.sync.dma_start(out=li[:], in_=ins[f"l2idx_{r}"][:])
                    lco = p3.tile([P, S2], bf16, name=f"lc_{r}", tag="lc")
                    nc.sync.dma_start(out=lco[:], in_=ins[f"l2coef_{r}"][:])
                    nc.gpsimd.ap_gather(
                        out_ap=go2[:, 0:S2, :], in_ap=tab2[:, :, :],
                        idxs_ap=li[:, :], channels=P, num_elems=NE1,
                        d=2, num_idxs=S2)
                    nc.vector.tensor_tensor(
                        out=go2[:, 0:S2, :], in0=go2[:, 0:S2, :],
                        in1=lco[:, :, None].to_broadcast([P, S2, 2]),
                        op=AL.mult)
                    pr_t = gpool.tile([P, nwin2, 2], f32, name=f"pr_{r}")
                    j0 = 0
                    for w in range(nwin2):
                        lw = int(seg[w])
                        nc.vector.tensor_reduce(
                            out=pr_t[:, w, :],
                            in_=go2[:, j0:j0 + lw, :].rearrange(
                                "p l c -> p c l"),
                            op=AL.add, axis=mybir.AxisListType.X)
                        j0 += lw
                    nc.sync.dma_start(
                        out=prank[r][:nwin2 * P, :].rearrange(
                            "(p k) c -> p k c", p=P),
                        in_=pr_t[:, :, :])
                    pr_ts.append(pr_t)

            # ---- phase 4: realign graphs (per-column), sum, bias, out
            with tc.tile_pool(name="p4", bufs=2) as p4:
                osum = gpool.tile([P, nwin2, 2], f32, name="osum")
                for r in range(R):
                    gp_t = p4.tile([P, nwin2], i32, name=f"gp_{r}", tag="gp")
                    nc.sync.dma_start(out=gp_t[:], in_=ins[f"gpos_{r}"][:])
                    gr = p4.tile([P, nwin2, 2], f32, name=f"gr_{r}", tag="gr")
                    for c0 in range(nwin2):
                        nc.gpsimd.indirect_dma_start(
                            out=gr[:, c0, :], out_offset=None, in_=prank[r][:],
                            in_offset=bass.IndirectOffsetOnAxis(
                                ap=gp_t[:, c0:c0 + 1], axis=0))
                    if r == 0:
                        nc.vector.tensor_copy(out=osum[:, :, :], in_=gr[:, :, :])
                    else:
                        nc.vector.tensor_add(out=osum[:, :, :],
                                             in0=osum[:, :, :], in1=gr[:, :, :])
                nc.sync.dma_start(
                    out=out_part.rearrange("(k p) c -> p k c", p=P),
                    in_=osum[:, :, :])
                b2all = p4.tile([P, R * 16], f32, name="b2all")
                nc.sync.dma_start(out=b2all[:], in_=b2b[:, :])
                b2s = p4.tile([P, 16], f32, name="b2s")
                nc.vector.tensor_reduce(
                    out=b2s[:], in_=b2all[:].rearrange("p (r f) -> p f r", r=R),
                    op=AL.add, axis=mybir.AxisListType.X)
                b2d = nc.dram_tensor("b2s_dram", [16], f32, kind="Internal").ap()
                nc.sync.dma_start(out=b2d[None, :], in_=b2s[0:1, :])
                b2col = p4.tile([16, 1], f32, name="b2col")
                nc.sync.dma_start(out=b2col[:], in_=b2d[:, None])
                bo_ps = psum.tile([1, 2], f32, name="bo_ps")
                wc2 = p4.tile([16, 2], f32, name="wc2")
                nc.sync.dma_start(out=wc2[:], in_=Wc[:, :])
                nc.tensor.matmul(out=bo_ps[:], lhsT=b2col[:], rhs=wc2[:],
                                 start=True, stop=True)
                bc_sb = p4.tile([1, 2], f32, name="bc_sb")
                nc.sync.dma_start(out=bc_sb[:], in_=bc[None, :])
                bo_sb = p4.tile([1, 2], f32, name="bo_sb")
                nc.vector.tensor_add(out=bo_sb[:], in0=bo_ps[:], in1=bc_sb[:])
                nc.sync.dma_start(out=bias_out[:, :], in_=bo_sb[:])
    nc.compile()
    return nc


def kernel(feat, src, dst, ew, graph_ids, W1, b1, W2, b2, Wc, bc):
    per_core, meta = _prep(feat, src, dst, ew, graph_ids)
    nc = _build_program(meta)
    w1f = np.ascontiguousarray(W1, dtype=np.float32) \
        .transpose(2, 0, 1).reshape(-1)  # [f, (r, c)] f-major for phase-2 FMA
    b1f = np.ascontiguousarray(b1, dtype=np.float32).reshape(-1)
    b2f = np.ascontiguousarray(b2, dtype=np.float32).reshape(-1)
    common = {
        "feat": np.ascontiguousarray(feat, dtype=np.float32),
        "w1p": np.tile(w1f, (P, 1)),
        "b1b": np.tile(b1f, (P, 1)),
        "W2": np.ascontiguousarray(W2, dtype=np.float32),
        "b2b": np.tile(b2f, (P, 1)),
        "Wc": np.ascontiguousarray(Wc, dtype=np.float32),
        "bc": np.ascontiguousarray(bc, dtype=np.float32),
    }
    in_maps = [{**common, **per_core[c]} for c in range(NCORES)]
    import os as _os
    import time as _t
    _t0 = _t.perf_counter()
    res = bass_utils.run_bass_kernel_spmd(
        nc, in_maps, core_ids=list(range(NCORES)),
        tmpdir=_os.environ.get("K_TRACE_DIR") or None)
    global LAST_EXEC_NS, LAST_TRACE
    LAST_EXEC_NS = int((_t.perf_counter() - _t0) * 1e9)
    if res.exec_time_ns:
        LAST_EXEC_NS = int(res.exec_time_ns)
    LAST_TRACE = res.instructions_and_trace[1] if res.instructions_and_trace else None
    out = np.zeros((B, 2), dtype=np.float32)
    for c in range(NCORES):
        out += res.results[c]["out_part"]
    out += res.results[0]["bias_out"][0]
    return out
